# revision 36
# baseline (speedup 1.0000x reference)
"""Causal self-attention (B=4, T=2048, C=2048, H=16, rope) on 8 trn2 cores.

Sharding: core c handles batch b = c//2 and head-group g = c%2 (8 heads).

All matmuls run in bf16 (stationary loads hit FWL, so N=512 matmuls run at
the 216 ns streaming floor instead of f32r's ~253 ns). q/k/v never leave
SBUF: phase 1 computes them head-major so phase 2 of head h-1 overlaps
phase 1 of head h, with emission interleaved at (kind, ts)-chain x
(scores/attn@V)-unit granularity so exp's ACT latency hides under qkv
matmuls.

  phase 1 (per head, 12 chains): qkvT = W^T x^T with x fully SBUF-resident
    in bf16; rope applied on q/k straight out of PSUM (4 DVE ops,
    sign-vector trick) writing bf16 into SBUF q/k tiles; v copied to bf16
    and PE-transposed into the va tile (ones column appended for the
    softmax denominator).
  phase 2 (per head): scoresT = k_tile^T q (bf16, N=512, single-bank PSUM
    tiles in a 4-deep ring so score matmuls don't stall on exp), exp+scale
    fused on ACT -> bf16, causal mask multiply on diagonal tiles only,
    attn@V with the ones column so the denominator falls out of the same
    matmul, per-partition reciprocal normalize, PE-transpose y back to
    [d, t] before the gather so phase 4 needs no transpose.
  phase 3: per-head pairwise AllGather of y^T (8 small collectives that
    pipeline behind each head's completion; nothing before phase 4 waits
    on the partner core).
  phase 4: out[t, f-half] = y Wp^T in bf16, accumulation ordered so the
    last-arriving heads' tiles come last.

Host pre-tiles all weights in bf16 (>=4KB contiguous per partition) and
pre-permutes q/k weight rows so rope pairs (2m, 2m+1) land on partitions
(m, 64+m).

Known pitfalls baked into this design: dma_start_transpose is only
reliable DRAM -> contiguous full SBUF tile, and its ucode generation
costs ~3 us on the issuing sequencer while waiting inline on the source's
semaphore (head-of-line blocking) — hence PE transposes here.
"""
import sys

sys.path.insert(0, "/opt/trn_rl_repo")

import numpy as np
import ml_dtypes

import concourse.bass as bass
import concourse.tile as tile
from concourse import bacc, mybir
from concourse import bass_utils

F32 = mybir.dt.float32
BF16 = mybir.dt.bfloat16
AF = mybir.ActivationFunctionType
ALU = mybir.AluOpType
BF16NP = ml_dtypes.bfloat16

B, T, C = 4, 2048, 2048
NH, D = 16, 128
HL = 8              # heads per core
NCT = C // 128      # 16 c-tiles
NTT = T // 128      # 16 t-tiles
SCALE = 1.0 / np.sqrt(D)
RG = [[0, 1], [2, 3], [4, 5], [6, 7]]


def _interleave(units_a, units_b):
    """Round-robin emit closures from two lists, proportionally."""
    na, nb = len(units_a), len(units_b)
    ia = ib = 0
    while ia < na or ib < nb:
        if ib >= nb or (ia < na and ia * nb <= ib * na):
            units_a[ia]()
            ia += 1
        else:
            units_b[ib]()
            ib += 1


def _build():
    nc = bacc.Bacc("TRN2", target_bir_lowering=False, debug=False, num_devices=8)
    xT = nc.dram_tensor("xT", [C, T], BF16, kind="ExternalInput").ap()
    Wall = nc.dram_tensor("Wall", [3 * HL, 128, C], BF16, kind="ExternalInput").ap()
    WpT = nc.dram_tensor("WpT", [128, NCT, C // 2], BF16, kind="ExternalInput").ap()
    cos2 = nc.dram_tensor("cos2", [128, T], F32, kind="ExternalInput").ap()
    sin1 = nc.dram_tensor("sin1", [64, T], F32, kind="ExternalInput").ap()
    sgn = nc.dram_tensor("sgn", [128, 1], F32, kind="ExternalInput").ap()
    mask4 = nc.dram_tensor("mask4", [128, 4, 512], BF16, kind="ExternalInput").ap()
    ident = nc.dram_tensor("ident", [128, 128], BF16, kind="ExternalInput").ap()
    out = nc.dram_tensor("out", [T, C // 2], F32, kind="ExternalOutput").ap()

    with tile.TileContext(nc) as tc:
        with tc.tile_pool(name="dram", bufs=1, space="DRAM") as dram, \
             tc.tile_pool(name="const", bufs=1) as cpool:
            yg_in = [dram.tile([128, T], BF16, name=f"yg_in{h}") for h in range(HL)]
            yg_out = [dram.tile([2, 128, T], BF16, name=f"yg_out{h}")
                      for h in range(HL)]

            m4_sb = cpool.tile([128, 4, 512], BF16)
            nc.sync.dma_start(m4_sb[:], mask4)
            id_sb = cpool.tile([128, 128], BF16)
            nc.sync.dma_start(id_sb[:], ident)
            sg_sb = cpool.tile([128, 1], F32)
            nc.sync.dma_start(sg_sb[:], sgn)
            c2_sb = cpool.tile([128, T], F32)
            nc.gpsimd.dma_start(c2_sb[:], cos2)
            s1_sb = cpool.tile([64, T], F32)
            nc.gpsimd.dma_start(s1_sb[:], sin1)

            # Pool allocation order is stack-discipline: phase-2 pools live
            # longest, then phase-1 pools and x, which release mid-stream so
            # the phase-4 pools can reuse their SBUF.
            p2 = {}
            p2["qk"] = tc.alloc_tile_pool(name="p2qk", bufs=6)   # q+k, 3 heads
            p2["va"] = tc.alloc_tile_pool(name="p2va", bufs=3)
            p2["eb"] = tc.alloc_tile_pool(name="p2eb", bufs=18)
            p2["yn"] = tc.alloc_tile_pool(name="p2yn", bufs=4)
            p2["rc"] = tc.alloc_tile_pool(name="p2rc", bufs=4)
            p2["yts"] = tc.alloc_tile_pool(name="p2yts", bufs=2)
            # Single-k-tile score tiles (1 bank each) in a 4-deep ring keep
            # the score matmuls from stalling on exp's ACT latency.
            p2["sp"] = tc.alloc_tile_pool(name="p2sp", bufs=4, space="PSUM")
            p2["yp"] = tc.alloc_tile_pool(name="p2yp", bufs=2, space="PSUM")
            p1 = {}
            p1["w"] = tc.alloc_tile_pool(name="p1w", bufs=3)
            p1["ab"] = tc.alloc_tile_pool(name="p1ab", bufs=2)
            p1["vb"] = tc.alloc_tile_pool(name="p1vb", bufs=2)
            p1["ps"] = tc.alloc_tile_pool(name="p1ps", bufs=2, space="PSUM")
            xp = tc.alloc_tile_pool(name="p1x", bufs=1)

            # Head 0's weight tiles load before x so the first matmul chain
            # can start as soon as the first x tiles land.
            wt_pre = {}
            for wi in range(3):
                wt = p1["w"].tile([128, C], BF16, name="wt")
                nc.sync.dma_start(wt[:], Wall[wi])
                wt_pre[wi] = wt

            # x fully SBUF-resident in bf16 (64 KB/partition).
            xsb = []
            for ct in range(NCT):
                xt_ = xp.tile([128, T], BF16, name=f"x{ct}")
                (nc.sync if ct % 2 == 0 else nc.scalar).dma_start(
                    xt_[:], xT[ct * 128:(ct + 1) * 128, :])
                xsb.append(xt_)

            st = {}    # per-head live tiles: (qt, kt, va, yts)
            wts = {}   # (h, wi) -> weight tile

            # ---------- phase 1 unit: one (head, kind, ts) chain ----------
            def p1_units(h):
                def prologue():
                    qt = p2["qk"].tile([128, T], BF16, name="qt")
                    kt = p2["qk"].tile([128, T], BF16, name="kt")
                    va = p2["va"].tile([128, NTT, 129], BF16, name="va")
                    nc.vector.memset(va[:, :, 128:129], 1.0)
                    yts = p2["yts"].tile([128, NTT, 128], BF16, name="yts")
                    st[h] = (qt, kt, va, yts)

                def do_w(wi, ts):
                    # wi: 0 -> v, 1 -> q, 2 -> k
                    def go():
                        if wi == 0 and ts == 0:
                            prologue()
                        qt, kt, va, yts = st[h]
                        if ts == 0:
                            if h == 0:
                                wts[(h, wi)] = wt_pre[wi]
                            else:
                                wt = p1["w"].tile([128, C], BF16, name="wt")
                                (nc.sync if wi % 2 == 0 else nc.scalar
                                 ).dma_start(wt[:], Wall[3 * h + wi])
                                wts[(h, wi)] = wt
                        wt = wts[(h, wi)]
                        t0 = ts * 512
                        ps = p1["ps"].tile([128, 512], F32, name="qkvps")
                        for ct in range(NCT):
                            nc.tensor.matmul(
                                ps[:], wt[:, ct * 128:(ct + 1) * 128],
                                xsb[ct][:, t0:t0 + 512],
                                start=(ct == 0), stop=(ct == NCT - 1))
                        if wi == 0:
                            vb = p1["vb"].tile([128, 512], BF16, name="vb")
                            nc.scalar.copy(vb[:], ps[:])
                            for q in range(4):
                                vtp = p1["ps"].tile([128, 128], BF16,
                                                    name="vtp", tag="qkvps")
                                nc.tensor.transpose(
                                    vtp[:], vb[:, q * 128:(q + 1) * 128],
                                    id_sb[:])
                                nc.vector.tensor_copy(
                                    va[:, 4 * ts + q, 0:128], vtp[:])
                        else:
                            a_t = p1["ab"].tile([128, 512], F32, name="a_t")
                            nc.vector.tensor_mul(
                                a_t[:], ps[:], c2_sb[:, t0:t0 + 512])
                            b_t = p1["ab"].tile([128, 512], F32, name="b_t")
                            nc.vector.tensor_mul(
                                b_t[0:64, :], ps[64:128, :],
                                s1_sb[:, t0:t0 + 512])
                            nc.vector.tensor_mul(
                                b_t[64:128, :], ps[0:64, :],
                                s1_sb[:, t0:t0 + 512])
                            dst = qt if wi == 1 else kt
                            nc.vector.scalar_tensor_tensor(
                                dst[:, t0:t0 + 512], b_t[:], sg_sb[:],
                                a_t[:], op0=ALU.mult, op1=ALU.add)
                    return go

                return [do_w(wi, ts) for wi in range(3) for ts in range(4)]

            # ---------- phase 2 units: scores / attn@V split per chunk ----
            def p2_units(h):
                ebs_store = {}

                def sc(Q):
                    def go():
                        qt, kt, va, yts = st[h]
                        ebs = []
                        for j in range(4 * Q + 4):
                            sp = p2["sp"].tile([128, 512], F32, name="sp")
                            nc.tensor.matmul(
                                sp[:],
                                kt[:, j * 128:(j + 1) * 128],
                                qt[:, Q * 512:(Q + 1) * 512],
                                start=True, stop=True)
                            eb = p2["eb"].tile([128, 512], BF16, name="eb")
                            nc.scalar.activation(
                                eb[:], sp[:], AF.Exp, scale=float(SCALE))
                            if j >= 4 * Q:
                                nc.vector.tensor_mul(
                                    eb[:], eb[:], m4_sb[:, j - 4 * Q, :])
                            ebs.append(eb)
                        ebs_store[Q] = ebs
                    return go

                def av(Q):
                    def go():
                        qt, kt, va, yts = st[h]
                        ebs = ebs_store.pop(Q)
                        for ql in range(4):
                            qt_i = Q * 4 + ql
                            yp = p2["yp"].tile([128, 129], F32, name="yp")
                            for j in range(qt_i + 1):
                                nc.tensor.matmul(
                                    yp[:],
                                    ebs[j][:, ql * 128:(ql + 1) * 128],
                                    va[:, j, :],
                                    start=(j == 0), stop=(j == qt_i))
                            rc = p2["rc"].tile([128, 1], F32, name="rc")
                            nc.vector.reciprocal(rc[:], yp[:, 128:129])
                            yn = p2["yn"].tile([128, 128], BF16, name="yn")
                            nc.vector.tensor_scalar_mul(
                                yn[:], yp[:, 0:128], rc[:])
                            ytp = p2["yp"].tile([128, 128], BF16,
                                                name="ytp", tag="yp")
                            nc.tensor.transpose(ytp[:], yn[:], id_sb[:])
                            nc.vector.tensor_copy(yts[:, qt_i, :], ytp[:])
                    return go

                def epilogue():
                    yts = st[h][3]
                    nc.sync.dma_start(
                        yg_in[h].rearrange("d (tt t) -> d tt t", t=128),
                        yts[:])
                    nc.gpsimd.collective_compute(
                        "AllGather", ALU.bypass,
                        ins=[yg_in[h][:].opt()], outs=[yg_out[h][:].opt()],
                        replica_groups=RG)
                    del st[h]

                units = []
                for Q in range(4):
                    units.append(sc(Q))
                    units.append(av(Q))
                units.append(epilogue)
                return units

            # ---------- emit: head-major pipeline with 1-head lookahead ----
            for u in p1_units(0):
                u()
            for h in range(1, HL):
                _interleave(p1_units(h), p2_units(h - 1))

            # Release x + phase-1 pools now (last phase-1 use is behind us)
            # so Wp and gathered-y loads can start during head 7's phase 2,
            # reusing the freed SBUF as soon as its readers drain.
            xp.release()
            for pool in list(p1.values())[::-1]:
                pool.release()

            wp_pool = tc.alloc_tile_pool(name="p4w", bufs=1)
            wp = wp_pool.tile([128, NCT, C // 2], BF16)
            nc.scalar.dma_start(wp[:], WpT)
            yf_pool = tc.alloc_tile_pool(name="p4y", bufs=1)
            yfs = [yf_pool.tile([128, 2, NTT, 128], BF16, name=f"yf{h}")
                   for h in range(HL)]

            def load_yf(h, eng):
                eng.dma_start(
                    yfs[h][:], yg_out[h][:].rearrange(
                        "r d (tt t) -> d r tt t", t=128))

            # Heads 0-4's gathers finished long ago: safe to preload on the
            # scalar queue without risking an inline-semaphore stall ahead of
            # head 7's exps. Heads 5-6 go on gpsimd where a wait is harmless.
            for h in range(5):
                load_yf(h, nc.scalar)
            load_yf(5, nc.gpsimd)
            load_yf(6, nc.gpsimd)

            for u in p2_units(HL - 1):
                u()
            load_yf(7, nc.sync)

            # Head 7's gather rides on cross-core skew (~25-40 us): release
            # the phase-2 PSUM pools and run heads 0-6's accumulation for the
            # first 8 output tiles (stop=False) inside that window, then
            # finish those banks with head 7's two tiles once it lands.
            p2["yp"].release()
            p2["sp"].release()
            ct_A = [r * 8 + h for h in range(HL - 1) for r in range(2)]
            ct_B = [7, 15]
            ct_order = ct_A + ct_B
            tiles = [(tt, fc) for tt in range(NTT) for fc in range(2)]
            NW = 8

            # ---------- phase 4: projection ----------
            with tc.tile_pool(name="p4o", bufs=2) as o_pool, \
                 tc.tile_pool(name="p4ps", bufs=NW, space="PSUM") as pp_pool:
                def chain(pp, tt, fc, cts, start, stop):
                    for i, ct in enumerate(cts):
                        r, h = ct // 8, ct % 8
                        nc.tensor.matmul(
                            pp[:],
                            yfs[h][:, r, tt, :],
                            wp[:, ct, fc * 512:(fc + 1) * 512],
                            start=(start and i == 0),
                            stop=(stop and i == len(cts) - 1))

                def finish(pp, tt, fc):
                    ob = o_pool.tile([128, 512], F32, name="ob")
                    nc.vector.tensor_copy(ob[:], pp[:])
                    nc.sync.dma_start(
                        out[tt * 128:(tt + 1) * 128,
                            fc * 512:(fc + 1) * 512], ob[:])

                pps = []
                for tt, fc in tiles[:NW]:
                    pp = pp_pool.tile([128, 512], F32, name="pp")
                    chain(pp, tt, fc, ct_A, start=True, stop=False)
                    pps.append(pp)
                for (tt, fc), pp in zip(tiles[:NW], pps):
                    chain(pp, tt, fc, ct_B, start=False, stop=True)
                    finish(pp, tt, fc)
                for tt, fc in tiles[NW:]:
                    pp = pp_pool.tile([128, 512], F32, name="pp")
                    chain(pp, tt, fc, ct_order, start=True, stop=True)
                    finish(pp, tt, fc)
            yf_pool.release()
            wp_pool.release()
            for name in ("yts", "rc", "yn", "eb", "va", "qk"):
                p2[name].release()
    nc.compile()
    return nc


_NC = None


def _get_nc():
    global _NC
    if _NC is None:
        _NC = _build()
    return _NC


def _rope_tables():
    inv_freq = (1.0 / (10000.0 ** (np.arange(0, D, 2, dtype=np.float32) / D)))
    t = np.arange(T, dtype=np.float32)
    freqs = np.outer(t, inv_freq).astype(np.float32)      # [T, 64]
    cos = np.cos(freqs).T                                 # [64, T]
    sin = np.sin(freqs).T
    cos2 = np.concatenate([cos, cos], 0).astype(np.float32)
    sin1 = np.ascontiguousarray(sin.astype(np.float32))
    return cos2, sin1


def _tile_w(Wt):
    """[128 r, 2048 c] weight tile -> [128 c_lo, 2048 (ct r)] layout."""
    return np.ascontiguousarray(
        Wt.T.reshape(NCT, 128, 128).transpose(1, 0, 2).reshape(128, C))


def make_in_maps(x, W_attn, W_proj):
    perm = np.concatenate([np.arange(0, D, 2), np.arange(1, D, 2)])
    cos2, sin1 = _rope_tables()
    sgn = np.concatenate([-np.ones((64, 1)), np.ones((64, 1))]).astype(np.float32)
    p_i = np.arange(128)[:, None, None]
    jj_i = np.arange(4)[None, :, None]
    c_i = np.arange(512)[None, None, :]
    mask4 = (c_i >= p_i + 128 * jj_i).astype(BF16NP)

    in_maps = []
    for core in range(8):
        b, g = core // 2, core % 2
        tiles = []
        for h in range(HL):
            hg = g * HL + h
            tiles.append(_tile_w(W_attn[2 * C + hg * D:2 * C + (hg + 1) * D]))
            tiles.append(_tile_w(W_attn[hg * D:(hg + 1) * D][perm]))
            tiles.append(_tile_w(W_attn[C + hg * D:C + (hg + 1) * D][perm]))
        Wall = np.stack(tiles, 0).astype(BF16NP)
        WpT = np.ascontiguousarray(
            W_proj[g * (C // 2):(g + 1) * (C // 2), :].T
        ).reshape(NCT, 128, C // 2).transpose(1, 0, 2)
        in_maps.append({
            "xT": np.ascontiguousarray(x[b].T).astype(BF16NP),
            "Wall": Wall,
            "WpT": np.ascontiguousarray(WpT).astype(BF16NP),
            "cos2": cos2, "sin1": sin1, "sgn": sgn,
            "mask4": mask4, "ident": np.eye(128, dtype=BF16NP),
        })
    return in_maps


def _assemble(results):
    out = np.empty((B, T, C), dtype=np.float32)
    for core in range(8):
        b, g = core // 2, core % 2
        out[b][:, g * (C // 2):(g + 1) * (C // 2)] = results[core]["out"]
    return out


def run(x, W_attn, W_proj, **spmd_kwargs):
    nc = _get_nc()
    in_maps = make_in_maps(np.asarray(x, dtype=np.float32),
                           np.asarray(W_attn, dtype=np.float32),
                           np.asarray(W_proj, dtype=np.float32))
    res = bass_utils.run_bass_kernel_spmd(
        nc, in_maps, core_ids=list(range(8)), **spmd_kwargs)
    return _assemble(res.results), res


def kernel(x, W_attn, W_proj):
    out, _ = run(x, W_attn, W_proj)
    return out
